# revision 4
# baseline (speedup 1.0000x reference)
"""Bass/Trainium2 kernel for nn_AtomScheduler (per-row right-shift placement).

out[b,c,t] = items[b,c,t-s] for t >= s else 0, with s = floor(positions[b,c]*N).

Strategy (pure data parallel over 8 NeuronCores, 2 batches per core = 128 rows):
- Host pads each row with N leading zeros: padded[r] = [zeros_N | items_r] (2N).
- Device computes s per row from positions, then for each output chunk does an
  indirect (gathering) DMA: row r's output chunk j is the CH-element window
  starting at flat offset r*2N + N - s_r + j*CH of the padded input -- the
  window covers [zeros tail | row head], which IS the shifted output.
- Chunks that are entirely in the zero region (s_r >= (j+1)*CH) skip the DMA
  read entirely: their index is pushed past bounds_check so the gather drops
  the row, and a prior SBUF memset supplies the zeros. This cuts DRAM read
  traffic roughly in half on average.
"""

import numpy as np

import concourse.bacc as bacc
import concourse.bass as bass
import concourse.mybir as mybir
import concourse.tile as tile
from concourse.bass_utils import run_bass_kernel_spmd

B, C, N = 16, 64, 32768
P = 128          # rows per core (= partitions): 2 batches x 64 clips
NCORES = 8
CH = 4096        # output chunk (free-dim) per pipeline step
NCHUNK = N // CH
ROW2 = 2 * N     # padded row stride
TOT = P * ROW2   # padded flat length per core
BIG = 1 << 28    # index offset that pushes a row past bounds_check

_cached = {}


def _build():
    nc = bacc.Bacc(None, target_bir_lowering=False, debug=False)
    f32 = mybir.dt.float32
    i32 = mybir.dt.int32
    op = mybir.AluOpType

    # flat padded items, viewed 2D so the DMA access pattern stays legal;
    # axis=1 of this view gives indirect-offset coefficient 1 (raw elements).
    items_pad = nc.declare_dram_parameter("items_pad", [TOT // 512, 512], f32, isOutput=False)
    positions = nc.declare_dram_parameter("positions", [P], f32, isOutput=False)
    out = nc.declare_dram_parameter("out", [P, N], f32, isOutput=True)

    with tile.TileContext(nc) as tc:
        with (
            tc.tile_pool(name="const", bufs=1) as cpool,
            tc.tile_pool(name="gbuf", bufs=4) as gpool,
        ):
            pos_t = cpool.tile([P, 1], f32)
            nc.sync.dma_start(out=pos_t[:, :1], in_=positions[:, None])

            # s_f = floor(pos * N) as an exact-integral f32, robust to either
            # rounding mode of the f32->i32 convert.
            s_raw = cpool.tile([P, 1], f32)
            nc.vector.tensor_scalar_mul(s_raw[:], pos_t[:], float(N))
            s_i0 = cpool.tile([P, 1], i32)
            nc.vector.tensor_copy(s_i0[:], s_raw[:])
            s_back = cpool.tile([P, 1], f32)
            nc.vector.tensor_copy(s_back[:], s_i0[:])
            err = cpool.tile([P, 1], f32)
            nc.vector.tensor_tensor(out=err[:], in0=s_back[:], in1=s_raw[:], op=op.is_gt)
            s_f = cpool.tile([P, 1], f32)
            nc.vector.tensor_tensor(out=s_f[:], in0=s_back[:], in1=err[:], op=op.subtract)

            # window starts for all chunks: base[r,j] = r*2N + N + j*CH (f32-exact)
            base_f = cpool.tile([P, NCHUNK], f32)
            base_i = cpool.tile([P, NCHUNK], i32)
            nc.gpsimd.iota(base_i[:], pattern=[[CH, NCHUNK]], base=N, channel_multiplier=ROW2)
            nc.vector.tensor_copy(base_f[:], base_i[:])
            idx_f = cpool.tile([P, NCHUNK], f32)
            nc.vector.tensor_scalar(
                out=idx_f[:], in0=base_f[:], scalar1=s_f[:, :1], scalar2=None,
                op0=op.subtract,
            )
            idx_i = cpool.tile([P, NCHUNK], i32)
            nc.vector.tensor_copy(idx_i[:], idx_f[:])

            # keep[r,j] = 1 iff chunk j has any data (s_r < (j+1)*CH), else 0.
            # Skipped rows get BIG added so bounds_check drops them.
            cend_i = cpool.tile([P, NCHUNK], i32)
            nc.gpsimd.iota(cend_i[:], pattern=[[CH, NCHUNK]], base=CH, channel_multiplier=0)
            cend_f = cpool.tile([P, NCHUNK], f32)
            nc.vector.tensor_copy(cend_f[:], cend_i[:])
            keep_i = cpool.tile([P, NCHUNK], i32)
            nc.vector.tensor_scalar(
                out=keep_i[:], in0=cend_f[:], scalar1=s_f[:, :1], scalar2=None,
                op0=op.is_gt,
            )
            skip_big = cpool.tile([P, NCHUNK], i32)
            nc.vector.tensor_scalar(
                out=skip_big[:], in0=keep_i[:], scalar1=1, scalar2=-BIG,
                op0=op.subtract, op1=op.mult,
            )
            idx_sk = cpool.tile([P, NCHUNK], i32)
            nc.vector.tensor_tensor(out=idx_sk[:], in0=idx_i[:], in1=skip_big[:], op=op.add)

            for j in range(NCHUNK):
                g = gpool.tile([P, CH], f32, tag="g")
                nc.vector.memset(g[:], 0.0)
                nc.gpsimd.indirect_dma_start(
                    out=g[:],
                    out_offset=None,
                    in_=items_pad[:],
                    in_offset=bass.IndirectOffsetOnAxis(ap=idx_sk[:, j:j + 1], axis=1),
                    bounds_check=TOT - 1,
                    oob_is_err=False,
                )
                nc.sync.dma_start(out=out[:, j * CH:(j + 1) * CH], in_=g[:])

    nc.compile()
    return nc


def kernel(items, positions, targets=None, **_):
    items = np.asarray(items, dtype=np.float32)
    positions = np.asarray(positions, dtype=np.float32)
    if "nc" not in _cached:
        _cached["nc"] = _build()
    nc = _cached["nc"]

    bpc = B // NCORES  # batches per core
    in_maps = []
    for i in range(NCORES):
        shard = items[i * bpc:(i + 1) * bpc].reshape(P, N)
        padded = np.zeros((P, ROW2), dtype=np.float32)
        padded[:, N:] = shard
        in_maps.append({
            "items_pad": padded.reshape(TOT // 512, 512),
            "positions": positions[i * bpc:(i + 1) * bpc].reshape(P).copy(),
        })

    res = run_bass_kernel_spmd(nc, in_maps, core_ids=list(range(NCORES)))
    _cached["exec_time_ns"] = res.exec_time_ns
    outs = [res.results[i]["out"].reshape(bpc, C, N) for i in range(NCORES)]
    return np.concatenate(outs, axis=0)


# revision 6
# speedup vs baseline: 1.2920x; 1.2920x over previous
"""Bass/Trainium2 kernel for nn_AtomScheduler (per-row right-shift placement).

out[b,c,t] = items[b,c,t-s] for t >= s else 0, with s = floor(positions[b,c]*N).

Strategy (pure data parallel over 8 NeuronCores, 2 batches per core = 128 rows):
- Host pads each row with N leading zeros: padded[r] = [zeros_N | items_r] (2N).
- Device computes s per row from positions, then for each output chunk does an
  indirect (gathering) DMA: row r's output chunk j is the CH-element window
  starting at flat offset r*2N + N - s_r + j*CH of the padded input -- the
  window covers [zeros tail | row head], which IS the shifted output.
- Chunks entirely in the zero region (s_r >= (j+1)*CH) are skipped on BOTH
  sides: their indices are pushed past bounds_check so the gather never reads
  and the indirect scatter never writes; output buffers start zeroed (the
  run_bass_kernel_spmd contract), so those regions remain zero. This cuts
  DRAM traffic roughly in half on average.
- Output is chunk-major (one DRAM tensor per chunk) so the per-chunk scatters
  have no WAW dependency; the host reassembles columns.
"""

import numpy as np

import concourse.bacc as bacc
import concourse.bass as bass
import concourse.mybir as mybir
import concourse.tile as tile
from concourse.bass_utils import run_bass_kernel_spmd

B, C, N = 16, 64, 32768
P = 128          # rows per core (= partitions): 2 batches x 64 clips
NCORES = 8
CH = 4096        # output chunk (free-dim) per pipeline step
NCHUNK = N // CH
ROW2 = 2 * N     # padded row stride
TOT = P * ROW2   # padded flat length per core
BIG = 1 << 28    # index offset that pushes a row past bounds_check

_cached = {}


def _build():
    nc = bacc.Bacc(None, target_bir_lowering=False, debug=False)
    f32 = mybir.dt.float32
    i32 = mybir.dt.int32
    op = mybir.AluOpType

    # flat padded items, viewed 2D so the DMA access pattern stays legal;
    # axis=1 of this view gives indirect-offset coefficient 1 (raw elements).
    items_pad = nc.declare_dram_parameter("items_pad", [TOT // 512, 512], f32, isOutput=False)
    positions = nc.declare_dram_parameter("positions", [P], f32, isOutput=False)
    outs = [
        nc.declare_dram_parameter(f"out{j}", [P * CH // 512, 512], f32, isOutput=True)
        for j in range(NCHUNK)
    ]

    with tile.TileContext(nc) as tc:
        with (
            tc.tile_pool(name="const", bufs=1) as cpool,
            tc.tile_pool(name="gbuf", bufs=4) as gpool,
        ):
            pos_t = cpool.tile([P, 1], f32)
            nc.sync.dma_start(out=pos_t[:, :1], in_=positions[:, None])

            # s_f = floor(pos * N) as an exact-integral f32, robust to either
            # rounding mode of the f32->i32 convert.
            s_raw = cpool.tile([P, 1], f32)
            nc.vector.tensor_scalar_mul(s_raw[:], pos_t[:], float(N))
            s_i0 = cpool.tile([P, 1], i32)
            nc.vector.tensor_copy(s_i0[:], s_raw[:])
            s_back = cpool.tile([P, 1], f32)
            nc.vector.tensor_copy(s_back[:], s_i0[:])
            err = cpool.tile([P, 1], f32)
            nc.vector.tensor_tensor(out=err[:], in0=s_back[:], in1=s_raw[:], op=op.is_gt)
            s_f = cpool.tile([P, 1], f32)
            nc.vector.tensor_tensor(out=s_f[:], in0=s_back[:], in1=err[:], op=op.subtract)

            # read-window starts for all chunks: base[r,j] = r*2N + N + j*CH
            base_f = cpool.tile([P, NCHUNK], f32)
            base_i = cpool.tile([P, NCHUNK], i32)
            nc.gpsimd.iota(base_i[:], pattern=[[CH, NCHUNK]], base=N, channel_multiplier=ROW2)
            nc.vector.tensor_copy(base_f[:], base_i[:])
            idx_f = cpool.tile([P, NCHUNK], f32)
            nc.vector.tensor_scalar(
                out=idx_f[:], in0=base_f[:], scalar1=s_f[:, :1], scalar2=None,
                op0=op.subtract,
            )
            idx_i = cpool.tile([P, NCHUNK], i32)
            nc.vector.tensor_copy(idx_i[:], idx_f[:])

            # keep[r,j] = 1 iff chunk j has any data (s_r < (j+1)*CH), else 0.
            # Skipped rows get BIG added so bounds_check drops them.
            cend_i = cpool.tile([P, NCHUNK], i32)
            nc.gpsimd.iota(cend_i[:], pattern=[[CH, NCHUNK]], base=CH, channel_multiplier=0)
            cend_f = cpool.tile([P, NCHUNK], f32)
            nc.vector.tensor_copy(cend_f[:], cend_i[:])
            keep_i = cpool.tile([P, NCHUNK], i32)
            nc.vector.tensor_scalar(
                out=keep_i[:], in0=cend_f[:], scalar1=s_f[:, :1], scalar2=None,
                op0=op.is_gt,
            )
            skip_big = cpool.tile([P, NCHUNK], i32)
            nc.vector.tensor_scalar(
                out=skip_big[:], in0=keep_i[:], scalar1=1, scalar2=-BIG,
                op0=op.subtract, op1=op.mult,
            )
            idx_sk = cpool.tile([P, NCHUNK], i32)
            nc.vector.tensor_tensor(out=idx_sk[:], in0=idx_i[:], in1=skip_big[:], op=op.add)

            # write-side indices within each chunk-major output: r*CH (+BIG if skip)
            rowoff_i = cpool.tile([P, 1], i32)
            nc.gpsimd.iota(rowoff_i[:], pattern=[[1, 1]], base=0, channel_multiplier=CH)
            widx_sk = cpool.tile([P, NCHUNK], i32)
            nc.vector.tensor_tensor(
                out=widx_sk[:], in0=skip_big[:],
                in1=rowoff_i[:, :1].to_broadcast([P, NCHUNK]), op=op.add,
            )

            for j in range(NCHUNK):
                g = gpool.tile([P, CH], f32, tag="g")
                nc.gpsimd.indirect_dma_start(
                    out=g[:],
                    out_offset=None,
                    in_=items_pad[:],
                    in_offset=bass.IndirectOffsetOnAxis(ap=idx_sk[:, j:j + 1], axis=1),
                    bounds_check=TOT - 1,
                    oob_is_err=False,
                )
                nc.gpsimd.indirect_dma_start(
                    out=outs[j][:],
                    out_offset=bass.IndirectOffsetOnAxis(ap=widx_sk[:, j:j + 1], axis=1),
                    in_=g[:],
                    in_offset=None,
                    bounds_check=P * CH - 1,
                    oob_is_err=False,
                )

    nc.compile()
    return nc


def kernel(items, positions, targets=None, **_):
    items = np.asarray(items, dtype=np.float32)
    positions = np.asarray(positions, dtype=np.float32)
    if "nc" not in _cached:
        _cached["nc"] = _build()
    nc = _cached["nc"]

    bpc = B // NCORES  # batches per core
    in_maps = []
    for i in range(NCORES):
        shard = items[i * bpc:(i + 1) * bpc].reshape(P, N)
        padded = np.zeros((P, ROW2), dtype=np.float32)
        padded[:, N:] = shard
        in_maps.append({
            "items_pad": padded.reshape(TOT // 512, 512),
            "positions": positions[i * bpc:(i + 1) * bpc].reshape(P).copy(),
        })

    res = run_bass_kernel_spmd(nc, in_maps, core_ids=list(range(NCORES)))
    _cached["exec_time_ns"] = res.exec_time_ns
    full = np.empty((B, C, N), dtype=np.float32)
    for i in range(NCORES):
        view = full[i * bpc:(i + 1) * bpc].reshape(P, N)
        for j in range(NCHUNK):
            view[:, j * CH:(j + 1) * CH] = res.results[i][f"out{j}"].reshape(P, CH)
    return full


# revision 8
# speedup vs baseline: 1.4334x; 1.1094x over previous
"""Bass/Trainium2 kernel for nn_AtomScheduler (per-row right-shift placement).

out[b,c,t] = items[b,c,t-s] for t >= s else 0, with s = floor(positions[b,c]*N).

Strategy (pure data parallel over 8 NeuronCores, 2 batches per core = 128 rows):
- Host pads each row with N leading zeros: padded[r] = [zeros_N | items_r] (2N).
- Device computes s per row from positions, then for each output chunk does an
  indirect (gathering) DMA: row r's output chunk j is the CH-element window
  starting at flat offset r*2N + N - s_r + j*CH of the padded input -- the
  window covers [zeros tail | row head], which IS the shifted output.
- Chunks entirely in the zero region (s_r >= (j+1)*CH) are skipped on BOTH
  sides: their indices are pushed past bounds_check so the gather never reads
  and the indirect scatter never writes; output buffers start zeroed (the
  run_bass_kernel_spmd contract), so those regions remain zero. This cuts
  DRAM traffic roughly in half on average.
- Output is chunk-major (one DRAM tensor per chunk) so the per-chunk scatters
  have no WAW dependency; the host reassembles columns.
- Rows are permuted across partitions on the host so each of the 16 SDMA
  engines (each hard-wired to 8 partitions) gets a balanced share of the
  data-dependent work; a wrow_off input carries each partition's original
  output slot.
"""

import numpy as np

import concourse.bacc as bacc
import concourse.bass as bass
import concourse.mybir as mybir
import concourse.tile as tile
from concourse.bass_utils import run_bass_kernel_spmd

B, C, N = 16, 64, 32768
P = 128          # rows per core (= partitions): 2 batches x 64 clips
NCORES = 8
CH = 4096        # output chunk (free-dim) per pipeline step
NCHUNK = N // CH
ROW2 = 2 * N     # padded row stride
TOT = P * ROW2   # padded flat length per core
BIG = 1 << 28    # index offset that pushes a row past bounds_check

_cached = {}


def _build():
    nc = bacc.Bacc(None, target_bir_lowering=False, debug=False)
    f32 = mybir.dt.float32
    i32 = mybir.dt.int32
    op = mybir.AluOpType

    # flat padded items, viewed 2D so the DMA access pattern stays legal;
    # axis=1 of this view gives indirect-offset coefficient 1 (raw elements).
    items_pad = nc.declare_dram_parameter("items_pad", [TOT // 512, 512], f32, isOutput=False)
    positions = nc.declare_dram_parameter("positions", [P], f32, isOutput=False)
    wrow_off = nc.declare_dram_parameter("wrow_off", [P], i32, isOutput=False)
    outs = [
        nc.declare_dram_parameter(f"out{j}", [P * CH // 512, 512], f32, isOutput=True)
        for j in range(NCHUNK)
    ]

    eng = nc.vector

    with tile.TileContext(nc) as tc:
        with (
            tc.tile_pool(name="const", bufs=1) as cpool,
            tc.tile_pool(name="gbuf", bufs=4) as gpool,
        ):
            pos_t = cpool.tile([P, 1], f32)
            nc.sync.dma_start(out=pos_t[:, :1], in_=positions[:, None])
            woff_t = cpool.tile([P, 1], i32)
            nc.sync.dma_start(out=woff_t[:, :1], in_=wrow_off[:, None])

            # s_f = floor(pos * N) as an exact-integral f32, robust to either
            # rounding mode of the f32->i32 convert.
            s_raw = cpool.tile([P, 1], f32)
            eng.tensor_scalar_mul(s_raw[:], pos_t[:], float(N))
            s_i0 = cpool.tile([P, 1], i32)
            eng.tensor_copy(s_i0[:], s_raw[:])
            s_back = cpool.tile([P, 1], f32)
            eng.tensor_copy(s_back[:], s_i0[:])
            err = cpool.tile([P, 1], f32)
            eng.tensor_tensor(out=err[:], in0=s_back[:], in1=s_raw[:], op=op.is_gt)
            s_f = cpool.tile([P, 1], f32)
            eng.tensor_tensor(out=s_f[:], in0=s_back[:], in1=err[:], op=op.subtract)

            # read-window starts for all chunks: base[r,j] = r*2N + N + j*CH
            base_f = cpool.tile([P, NCHUNK], f32)
            base_i = cpool.tile([P, NCHUNK], i32)
            nc.gpsimd.iota(base_i[:], pattern=[[CH, NCHUNK]], base=N, channel_multiplier=ROW2)
            eng.tensor_copy(base_f[:], base_i[:])
            idx_f = cpool.tile([P, NCHUNK], f32)
            eng.tensor_scalar(
                out=idx_f[:], in0=base_f[:], scalar1=s_f[:, :1], scalar2=None,
                op0=op.subtract,
            )
            idx_i = cpool.tile([P, NCHUNK], i32)
            eng.tensor_copy(idx_i[:], idx_f[:])

            # keep[r,j] = 1 iff chunk j has any data (s_r < (j+1)*CH), else 0.
            # Skipped rows get BIG added so bounds_check drops them.
            cend_i = cpool.tile([P, NCHUNK], i32)
            nc.gpsimd.iota(cend_i[:], pattern=[[CH, NCHUNK]], base=CH, channel_multiplier=0)
            cend_f = cpool.tile([P, NCHUNK], f32)
            eng.tensor_copy(cend_f[:], cend_i[:])
            keep_i = cpool.tile([P, NCHUNK], i32)
            eng.tensor_scalar(
                out=keep_i[:], in0=cend_f[:], scalar1=s_f[:, :1], scalar2=None,
                op0=op.is_gt,
            )
            skip_big = cpool.tile([P, NCHUNK], i32)
            eng.tensor_scalar(
                out=skip_big[:], in0=keep_i[:], scalar1=1, scalar2=-BIG,
                op0=op.subtract, op1=op.mult,
            )
            idx_sk = cpool.tile([P, NCHUNK], i32)
            eng.tensor_tensor(out=idx_sk[:], in0=idx_i[:], in1=skip_big[:], op=op.add)

            # write-side indices: original row slot offset (+BIG if skip)
            widx_sk = cpool.tile([P, NCHUNK], i32)
            eng.tensor_tensor(
                out=widx_sk[:], in0=skip_big[:],
                in1=woff_t[:, :1].to_broadcast([P, NCHUNK]), op=op.add,
            )

            for j in range(NCHUNK):
                g = gpool.tile([P, CH], f32, tag="g")
                nc.gpsimd.indirect_dma_start(
                    out=g[:],
                    out_offset=None,
                    in_=items_pad[:],
                    in_offset=bass.IndirectOffsetOnAxis(ap=idx_sk[:, j:j + 1], axis=1),
                    bounds_check=TOT - 1,
                    oob_is_err=False,
                )
                nc.gpsimd.indirect_dma_start(
                    out=outs[j][:],
                    out_offset=bass.IndirectOffsetOnAxis(ap=widx_sk[:, j:j + 1], axis=1),
                    in_=g[:],
                    in_offset=None,
                    bounds_check=P * CH - 1,
                    oob_is_err=False,
                )

    nc.compile()
    return nc


def _sdma_engine(p):
    """SDMA engine serving SBUF partition p (port swizzle)."""
    if p < 64:
        return 2 * ((p % 32) // 4)
    return 2 * (((p - 64) % 32) // 4) + 1


_ENGINE_PARTS = [[p for p in range(P) if _sdma_engine(p) == e] for e in range(16)]


def _balance_perm(s):
    """perm[p] = original row handled by partition p, balancing per-engine work."""
    kept = NCHUNK - np.minimum(s // CH, NCHUNK - 1)  # chunks moved per row
    order = np.argsort(-kept, kind="stable")
    esum = np.zeros(16, dtype=np.int64)
    eslots = [list(_ENGINE_PARTS[e]) for e in range(16)]
    perm = np.zeros(P, dtype=np.int64)
    for r in order:
        cands = [e for e in range(16) if eslots[e]]
        e = min(cands, key=lambda x: (esum[x], -len(eslots[x])))
        perm[eslots[e].pop()] = r
        esum[e] += kept[r]
    return perm


def _prep_core(shard, pos_shard):
    """shard: (P, N) f32, pos_shard: (P,) f32 -> in_map dict (+ perm)."""
    s = (pos_shard * N).astype(np.int32)
    perm = _balance_perm(s)
    padded = np.zeros((P, ROW2), dtype=np.float32)
    padded[:, N:] = shard[perm]
    return {
        "items_pad": padded.reshape(TOT // 512, 512),
        "positions": pos_shard[perm].copy(),
        "wrow_off": (perm * CH).astype(np.int32),
    }


def kernel(items, positions, targets=None, **_):
    items = np.asarray(items, dtype=np.float32)
    positions = np.asarray(positions, dtype=np.float32)
    if "nc" not in _cached:
        _cached["nc"] = _build()
    nc = _cached["nc"]

    bpc = B // NCORES  # batches per core
    in_maps = [
        _prep_core(
            items[i * bpc:(i + 1) * bpc].reshape(P, N),
            positions[i * bpc:(i + 1) * bpc].reshape(P),
        )
        for i in range(NCORES)
    ]

    res = run_bass_kernel_spmd(nc, in_maps, core_ids=list(range(NCORES)))
    _cached["exec_time_ns"] = res.exec_time_ns
    full = np.empty((B, C, N), dtype=np.float32)
    for i in range(NCORES):
        view = full[i * bpc:(i + 1) * bpc].reshape(P, N)
        for j in range(NCHUNK):
            view[:, j * CH:(j + 1) * CH] = res.results[i][f"out{j}"].reshape(P, CH)
    return full


# revision 9
# speedup vs baseline: 1.5132x; 1.0557x over previous
"""Bass/Trainium2 kernel for nn_AtomScheduler (per-row right-shift placement).

out[b,c,t] = items[b,c,t-s] for t >= s else 0, with s = floor(positions[b,c]*N).

Strategy (pure data parallel over 8 NeuronCores, 2 batches per core = 128 rows):
- Host pads each row with N leading zeros: padded[r] = [zeros_N | items_r] (2N).
- Device computes s per row from positions, then for each output chunk does an
  indirect (gathering) DMA: row r's output chunk j is the CH-element window
  starting at flat offset r*2N + N - s_r + j*CH of the padded input -- the
  window covers [zeros tail | row head], which IS the shifted output.
- Chunks entirely in the zero region (s_r >= (j+1)*CH) are skipped on BOTH
  sides: their indices are pushed past bounds_check so the gather never reads
  and the indirect scatter never writes; output buffers start zeroed (the
  run_bass_kernel_spmd contract), so those regions remain zero. This cuts
  DRAM traffic roughly in half on average.
- Output is chunk-major (one DRAM tensor per chunk) so the per-chunk scatters
  have no WAW dependency; the host reassembles columns.
- Rows are permuted across partitions on the host so each of the 16 SDMA
  engines (each hard-wired to 8 partitions) gets a balanced share of the
  data-dependent work; a wrow_off input carries each partition's original
  output slot.
"""

import numpy as np

import concourse.bacc as bacc
import concourse.bass as bass
import concourse.mybir as mybir
import concourse.tile as tile
from concourse.bass_utils import run_bass_kernel_spmd

B, C, N = 16, 64, 32768
P = 128          # rows per core (= partitions): 2 batches x 64 clips
NCORES = 8
CH = 4096        # output chunk (free-dim) per pipeline step
NCHUNK = N // CH
ROW2 = 2 * N     # padded row stride
TOT = P * ROW2   # padded flat length per core
BIG = 1 << 28    # index offset that pushes a row past bounds_check

_cached = {}


def _build():
    nc = bacc.Bacc(None, target_bir_lowering=False, debug=False)
    f32 = mybir.dt.float32
    i32 = mybir.dt.int32
    op = mybir.AluOpType

    # flat padded items, viewed 2D so the DMA access pattern stays legal;
    # axis=1 of this view gives indirect-offset coefficient 1 (raw elements).
    items_pad = nc.declare_dram_parameter("items_pad", [TOT // 512, 512], f32, isOutput=False)
    positions = nc.declare_dram_parameter("positions", [P], f32, isOutput=False)
    outs = [
        nc.declare_dram_parameter(f"out{j}", [P * CH // 512, 512], f32, isOutput=True)
        for j in range(NCHUNK)
    ]

    eng = nc.vector

    with tile.TileContext(nc) as tc:
        with (
            tc.tile_pool(name="const", bufs=1) as cpool,
            tc.tile_pool(name="gbuf", bufs=4) as gpool,
        ):
            pos_t = cpool.tile([P, 1], f32)
            nc.sync.dma_start(out=pos_t[:, :1], in_=positions[:, None])

            # s_f = floor(pos * N) as an exact-integral f32, robust to either
            # rounding mode of the f32->i32 convert.
            s_raw = cpool.tile([P, 1], f32)
            eng.tensor_scalar_mul(s_raw[:], pos_t[:], float(N))
            s_i0 = cpool.tile([P, 1], i32)
            eng.tensor_copy(s_i0[:], s_raw[:])
            s_back = cpool.tile([P, 1], f32)
            eng.tensor_copy(s_back[:], s_i0[:])
            err = cpool.tile([P, 1], f32)
            eng.tensor_tensor(out=err[:], in0=s_back[:], in1=s_raw[:], op=op.is_gt)
            s_f = cpool.tile([P, 1], f32)
            eng.tensor_tensor(out=s_f[:], in0=s_back[:], in1=err[:], op=op.subtract)

            # read-window starts for all chunks: base[r,j] = r*2N + N + j*CH
            base_f = cpool.tile([P, NCHUNK], f32)
            base_i = cpool.tile([P, NCHUNK], i32)
            nc.gpsimd.iota(base_i[:], pattern=[[CH, NCHUNK]], base=N, channel_multiplier=ROW2)
            eng.tensor_copy(base_f[:], base_i[:])
            idx_f = cpool.tile([P, NCHUNK], f32)
            eng.tensor_scalar(
                out=idx_f[:], in0=base_f[:], scalar1=s_f[:, :1], scalar2=None,
                op0=op.subtract,
            )
            idx_i = cpool.tile([P, NCHUNK], i32)
            eng.tensor_copy(idx_i[:], idx_f[:])

            # keep[r,j] = 1 iff chunk j has any data (s_r < (j+1)*CH), else 0.
            # Skipped rows get BIG added so bounds_check drops them.
            cend_i = cpool.tile([P, NCHUNK], i32)
            nc.gpsimd.iota(cend_i[:], pattern=[[CH, NCHUNK]], base=CH, channel_multiplier=0)
            cend_f = cpool.tile([P, NCHUNK], f32)
            eng.tensor_copy(cend_f[:], cend_i[:])
            keep_i = cpool.tile([P, NCHUNK], i32)
            eng.tensor_scalar(
                out=keep_i[:], in0=cend_f[:], scalar1=s_f[:, :1], scalar2=None,
                op0=op.is_gt,
            )
            skip_big = cpool.tile([P, NCHUNK], i32)
            eng.tensor_scalar(
                out=skip_big[:], in0=keep_i[:], scalar1=1, scalar2=-BIG,
                op0=op.subtract, op1=op.mult,
            )
            idx_sk = cpool.tile([P, NCHUNK], i32)
            eng.tensor_tensor(out=idx_sk[:], in0=idx_i[:], in1=skip_big[:], op=op.add)

            # write-side indices: partition-identity row offset (+BIG if skip);
            # the host un-permutes rows during reassembly.
            rowoff_i = cpool.tile([P, 1], i32)
            nc.gpsimd.iota(rowoff_i[:], pattern=[[1, 1]], base=0, channel_multiplier=CH)
            widx_sk = cpool.tile([P, NCHUNK], i32)
            eng.tensor_tensor(
                out=widx_sk[:], in0=skip_big[:],
                in1=rowoff_i[:, :1].to_broadcast([P, NCHUNK]), op=op.add,
            )

            for j in range(NCHUNK):
                g = gpool.tile([P, CH], f32, tag="g")
                nc.gpsimd.indirect_dma_start(
                    out=g[:],
                    out_offset=None,
                    in_=items_pad[:],
                    in_offset=bass.IndirectOffsetOnAxis(ap=idx_sk[:, j:j + 1], axis=1),
                    bounds_check=TOT - 1,
                    oob_is_err=False,
                )
                nc.gpsimd.indirect_dma_start(
                    out=outs[j][:],
                    out_offset=bass.IndirectOffsetOnAxis(ap=widx_sk[:, j:j + 1], axis=1),
                    in_=g[:],
                    in_offset=None,
                    bounds_check=P * CH - 1,
                    oob_is_err=False,
                )

    nc.compile()
    return nc


def _sdma_engine(p):
    """SDMA engine serving SBUF partition p (port swizzle)."""
    if p < 64:
        return 2 * ((p % 32) // 4)
    return 2 * (((p - 64) % 32) // 4) + 1


_ENGINE_PARTS = [[p for p in range(P) if _sdma_engine(p) == e] for e in range(16)]


def _balance_perm(s):
    """perm[p] = original row handled by partition p, balancing per-engine work."""
    kept = NCHUNK - np.minimum(s // CH, NCHUNK - 1)  # chunks moved per row
    cap = np.ones(16)
    cap[7] = cap[15] = 0.9  # SWDGE descriptor-ring port contention derate
    order = np.argsort(-kept, kind="stable")
    esum = np.zeros(16)
    eslots = [list(_ENGINE_PARTS[e]) for e in range(16)]
    perm = np.zeros(P, dtype=np.int64)
    for r in order:
        cands = [e for e in range(16) if eslots[e]]
        e = min(cands, key=lambda x: ((esum[x] + kept[r]) / cap[x], -len(eslots[x])))
        perm[eslots[e].pop()] = r
        esum[e] += kept[r]
    return perm


def _prep_core(shard, pos_shard):
    """shard: (P, N) f32, pos_shard: (P,) f32 -> (in_map dict, perm)."""
    s = (pos_shard * N).astype(np.int32)
    perm = _balance_perm(s)
    padded = np.zeros((P, ROW2), dtype=np.float32)
    padded[:, N:] = shard[perm]
    return {
        "items_pad": padded.reshape(TOT // 512, 512),
        "positions": pos_shard[perm].copy(),
    }, perm


def kernel(items, positions, targets=None, **_):
    items = np.asarray(items, dtype=np.float32)
    positions = np.asarray(positions, dtype=np.float32)
    if "nc" not in _cached:
        _cached["nc"] = _build()
    nc = _cached["nc"]

    bpc = B // NCORES  # batches per core
    prepped = [
        _prep_core(
            items[i * bpc:(i + 1) * bpc].reshape(P, N),
            positions[i * bpc:(i + 1) * bpc].reshape(P),
        )
        for i in range(NCORES)
    ]
    in_maps = [p[0] for p in prepped]
    perms = [p[1] for p in prepped]

    res = run_bass_kernel_spmd(nc, in_maps, core_ids=list(range(NCORES)))
    _cached["exec_time_ns"] = res.exec_time_ns
    full = np.empty((B, C, N), dtype=np.float32)
    for i in range(NCORES):
        view = full[i * bpc:(i + 1) * bpc].reshape(P, N)
        for j in range(NCHUNK):
            view[perms[i], j * CH:(j + 1) * CH] = res.results[i][f"out{j}"].reshape(P, CH)
    return full
